# revision 23
# baseline (speedup 1.0000x reference)
"""Trainium2 Bass kernel for AttentionBlock (B=8, C=256, L=2048), data-parallel
over batch across 8 NeuronCores.

Math (one batch per core, x: [C, L]):
    t^T   = w8^T x8            w8 = fp8(kappa M x),  M = Wq^T Wk,  kappa = 128*SCALE/ln2
    pT    = exp(t*ln2/128 + ux)  [m, l], keys m on partitions, fp8 direct from ACT
    denom = per-query sum of pT: cols 0:1024 via bf16 DVE accumulator + ones
            matmul, cols 1024:2048 via fp8 DoubleRow ones-matmuls in PSUM
    ctx   = vT^T pT in fp8 DoubleRow over chunk PAIRS (256 keys/instruction)
    out   = ctx * (1/denom) + (bf16(x) + bv)

Schedule (v10):
  - exp: 2 x 1024-wide ACT instructions per chunk writing fp8 pT directly;
    ACT paces phase 2 at ~2.26us/chunk
  - x8 loads ride FIVE dma queues (sync/scalar/gpsimd/vector/tensor) in
    256-col slices so all of x8 is resident by ~11.5us; xbf rides last on
    the scalar queue (coarse per-queue completion semaphores would
    otherwise stall x8 consumers behind the 1MB xbf transfer)
  - no separate w-projection PSUM pool: cols 0:1024 go through two score
    tile rotations before chunk 0; cols 1024:2048 borrow the ds2/ds3/ctx20
    banks as scratch during chunks 1-4 (their accumulations start at pairs
    3/2/1 and the missed pairs are caught up at the start of phase 3)
  - PSUM: scores 2x[P,1024] (4) + vp (1) + ds2/ds3 (2) + ctx(2,0) (1)
  - phase 3: catch-up mms, denominator finish, 7 remaining ctx tiles in
    fp8 DR with evictions interleaved; tail tiles' residual adds on DVE
    (gpsimd tensor_add is ~1.1us/512) and the last tile evicts in 4
    sub-slices across 3 dma queues
"""

import math
import numpy as np
import ml_dtypes

import concourse.bass as bass
import concourse.tile as tile
from concourse import bacc, mybir
from concourse.bass_utils import run_bass_kernel_spmd

B, C, L = 8, 256, 2048
P = 128                 # partitions
NMC = L // P            # 16 m-chunks (key blocks)
NPAIR = NMC // 2        # 8 key pairs (256 keys each)
NB = 512                # matmul moving free dim
SCALE = float(C) ** -0.5
LN2 = math.log(2.0)
KAPPA = 128.0 * SCALE / LN2     # scores t = kappa * s_raw (baked into mt8 on host)

F32 = mybir.dt.float32
BF16 = mybir.dt.bfloat16
F8 = mybir.dt.float8e4
DR = mybir.MatmulPerfMode.DoubleRow

# first phase-2 pair each accumulator covers (earlier pairs caught up in
# phase 3); constrained by when each tile is free of w-projection scratch
CTX20_P0, DS2_P0, DS3_P0 = 1, 3, 2

_COMPILED = None


def build_nc():
    nc = bacc.Bacc("TRN2", target_bir_lowering=False, debug=False, num_devices=8)

    x8_d = nc.dram_tensor("x8", [C, L], F8, kind="ExternalInput").ap()
    xbf_d = nc.dram_tensor("xbf", [C, L], BF16, kind="ExternalInput").ap()
    mt8_d = nc.dram_tensor("mt8", [C, C], F8, kind="ExternalInput").ap()
    wvu8_d = nc.dram_tensor("wvu8", [C, 272], F8, kind="ExternalInput").ap()
    bv_d = nc.dram_tensor("bv", [C, 1], F32, kind="ExternalInput").ap()
    out_d = nc.dram_tensor("out", [C, L], BF16, kind="ExternalOutput").ap()

    with tile.TileContext(nc) as tc:
        with (
            tc.tile_pool(name="const", bufs=1) as const,
            tc.tile_pool(name="data", bufs=1) as data,
            tc.tile_pool(name="evict", bufs=6) as evict,
        ):
            # ---- constants ----
            ones_bf = const.tile([P, NB], BF16)
            nc.vector.memset(ones_bf[:], 1.0)
            ones8 = const.tile([P, 2, P], F8)
            nc.gpsimd.memset(ones8[:], 1.0)
            tiny = const.tile([P, 4, 16], F32)

            x8 = data.tile([P, 2, L], F8, tag="x8", name="x8")
            xbf = [data.tile([P, L], BF16, tag=f"xbf{c}", name=f"xbf{c}")
                   for c in range(2)]
            mt8 = const.tile([P, 2, C], F8, tag="mt8")
            wvu8 = const.tile([P, 2, 272], F8, tag="wvu8")
            bv_sb = const.tile([P, 2, 1], F32, tag="bv")

            def x8_dma(c0, c1, eng):
                cols = slice(c0, c1)
                eng.dma_start(out=x8[:, :, cols],
                              in_=x8_d[:, cols].rearrange("(j p) l -> p j l",
                                                          p=P))

            # x8 in 256-col slices across the 3 dma-capable queues in
            # consumption order; weights early on scalar. bv/xbf descriptor
            # generation is deferred (traced after the w8 evicts) so it
            # doesn't occupy the ACT/sync engines during startup, and xbf
            # rides last on the scalar queue (coarse per-queue completion
            # semaphores would otherwise stall x8 consumers behind it).
            nc.scalar.dma_start(out=mt8[:],
                                in_=mt8_d.rearrange("(j p) o -> p j o", p=P))
            x8_dma(0, 256, nc.gpsimd)
            x8_dma(256, 512, nc.sync)
            nc.scalar.dma_start(out=wvu8[:],
                                in_=wvu8_d.rearrange("(j p) o -> p j o", p=P))
            x8_dma(512, 768, nc.gpsimd)
            x8_dma(768, 1024, nc.sync)
            x8_dma(1024, 1280, nc.scalar)
            x8_dma(1280, 1536, nc.gpsimd)
            x8_dma(1536, 1792, nc.sync)
            x8_dma(1792, 2048, nc.gpsimd)

            w8 = data.tile([P, 2, L], F8, tag="w8", name="w8")
            vT8 = data.tile([P, NPAIR, 2, C], F8, tag="vT8")
            pT8 = data.tile([P, NPAIR, 2, L], F8, tag="pT8")
            b_act = data.tile([P, NMC, 1], F32, tag="b_act")
            bv_late = data.tile([P, 2, 1], F32, tag="bv_late")
            dacc = data.tile([P, 1024], BF16, tag="dacc")
            recip = data.tile([P, L], F32, tag="recip")
            xr = [data.tile([P, L], BF16, tag=f"xr{c}", name=f"xr{c}")
                  for c in range(2)]

            with tc.tile_pool(name="psCL", bufs=1,
                              space=bass.MemorySpace.PSUM) as psCL:
                ctx20 = psCL.tile([P, NB], F32, tag="c20", name="c20", bufs=1)
                ds23 = [psCL.tile([P, NB], F32, tag=f"ds{q}", name=f"ds{q}",
                                  bufs=1) for q in (2, 3)]

                with tc.tile_pool(name="psB", bufs=1,
                                  space=bass.MemorySpace.PSUM) as psB:
                    # ---- warmup: ACT/DVE table loads + PE spin-up; matmul
                    # targets are overwritten later via start=True ----
                    nc.vector.memset(tiny[:, 0, :], 1.0)
                    nc.scalar.activation(out=tiny[:, 1, :], in_=tiny[:, 0, :],
                                         func=mybir.ActivationFunctionType.Exp,
                                         scale=1.0)
                    nc.vector.reciprocal_approx_fast(out=tiny[:, 2, :],
                                                     in_=tiny[:, 0, :])
                    for i in range(3):
                        nc.tensor.matmul((ds23 + [ctx20])[i][:],
                                         ones_bf[:, 0:P], ones_bf[:],
                                         start=True, stop=True)
                    nc.tensor.matmul(ds23[0][0:32, 0:16], ones8[:, :, 0:32],
                                     ones8[:, :, 0:16], start=True, stop=True,
                                     perf_mode=DR)

                    # ---- w projection cols 0:1024 through two score-tile
                    # rotations (w = kappa M x, DoubleRow, 256 deep) ----
                    wpA = [psB.tile([P, 1024], F32, tag="s", name=f"wpA{oc}",
                                    bufs=2) for oc in range(2)]

                    def wmm(dst, dcols, oc, xcols):
                        nc.tensor.matmul(dst[:, dcols],
                                         mt8[:, :, oc * P:(oc + 1) * P],
                                         x8[:, :, xcols],
                                         start=True, stop=True, perf_mode=DR)

                    # evicts trace right after their block's matmuls: the
                    # completion semaphores are coarse (threshold = all PE
                    # work traced so far), so an evict traced after ALL four
                    # matmuls waits for all four. ACT takes oc0, DVE oc1.
                    for b in range(2):
                        cols = slice(b * NB, (b + 1) * NB)
                        for oc in range(2):
                            wmm(wpA[oc], cols, oc, cols)
                        nc.scalar.copy(out=w8[:, 0, cols], in_=wpA[0][:, cols])
                        nc.vector.tensor_copy(out=w8[:, 1, cols],
                                              in_=wpA[1][:, cols])
                    # deferred descriptor generation (data needed in phase 3)
                    nc.scalar.dma_start(
                        out=bv_sb[:],
                        in_=bv_d.rearrange("(j p) o -> p j o", p=P))
                    nc.scalar.dma_start(out=xbf[0][:], in_=xbf_d[0:P, :])
                    nc.scalar.dma_start(out=xbf[1][:], in_=xbf_d[P:C, :])

                    # late w-projection (cols 1024:2048) borrows psCL banks as
                    # scratch: (tile, psum cols, w8 oc, x8 cols, mm chunk,
                    # cast chunk)
                    late_w = [
                        (ds23[0], 0, slice(1024, 1536), 1, 1),
                        (ds23[1], 1, slice(1024, 1536), 1, 2),
                        (ctx20, 0, slice(1536, 2048), 2, 3),
                        (ds23[0], 1, slice(1536, 2048), 4, 5),
                    ]

                    for mc in range(NMC):
                        pair, par = mc // 2, mc % 2
                        mrows = slice(mc * P, (mc + 1) * P)
                        # v/ux projection for this key chunk
                        vp = psB.tile([P, 272], F32, tag="vp", name="vp",
                                      bufs=1)
                        nc.tensor.matmul(
                            vp[:], x8[:, :, mrows], wvu8[:],
                            start=True, stop=True, perf_mode=DR)
                        # scores, two 1024-wide tiles (2 x 512 mms each)
                        s = [psB.tile([P, 1024], F32, tag="s", name="s",
                                      bufs=2) for _ in range(2)]
                        for h in range(2):
                            for ln in range(2):
                                q0 = h * 1024 + ln * NB
                                nc.tensor.matmul(
                                    s[h][:, ln * NB:(ln + 1) * NB],
                                    w8[:, :, mrows],
                                    x8[:, :, q0:q0 + NB],
                                    start=True, stop=True, perf_mode=DR)
                        # late w-projection matmuls scheduled for this chunk
                        for t, oc, xc, mmc, cc_ in late_w:
                            if mmc == mc:
                                wmm(t, slice(0, NB), oc, xc)
                        # catch-up mms for the pairs the accumulators missed
                        # while holding w-projection scratch; traced BEFORE
                        # this chunk's exps (the coarse ACT semaphore would
                        # otherwise make them — and the next chunk's PE work —
                        # wait for this chunk's exp)
                        if mc == 12:
                            nc.tensor.matmul(ds23[0][:], ones8[:],
                                             pT8[:, 0, :, 1024:1536],
                                             start=False, stop=False,
                                             perf_mode=DR)
                            nc.tensor.matmul(ds23[1][:], ones8[:],
                                             pT8[:, 0, :, 1536:2048],
                                             start=False, stop=False,
                                             perf_mode=DR)
                        elif mc == 13:
                            nc.tensor.matmul(ds23[0][:], ones8[:],
                                             pT8[:, 1, :, 1024:1536],
                                             start=False, stop=False,
                                             perf_mode=DR)
                            nc.tensor.matmul(ds23[1][:], ones8[:],
                                             pT8[:, 1, :, 1536:2048],
                                             start=False, stop=False,
                                             perf_mode=DR)
                        elif mc == 14:
                            nc.tensor.matmul(ds23[0][:], ones8[:],
                                             pT8[:, 2, :, 1024:1536],
                                             start=False, stop=False,
                                             perf_mode=DR)
                            nc.tensor.matmul(ctx20[:], vT8[:, 0, :, 0:P],
                                             pT8[:, 0, :, 1024:1536],
                                             start=False, stop=False,
                                             perf_mode=DR)
                        # per-key exp bias (ux); vT frees the vp bank
                        nc.vector.tensor_copy(out=b_act[:, mc, :],
                                              in_=vp[:, C:C + 1])
                        nc.vector.tensor_copy(out=vT8[:, pair, par, :],
                                              in_=vp[:, 0:C])
                        # exp -> fp8 pT, 1024 cols per instruction
                        for h in range(2):
                            nc.scalar.activation(
                                out=pT8[:, pair, par, h * 1024:(h + 1) * 1024],
                                in_=s[h][:],
                                func=mybir.ActivationFunctionType.Exp,
                                scale=LN2 / 128.0, bias=b_act[:, mc, :])
                        # late w-projection evictions (DVE, one per chunk)
                        for t, oc, xc, mmc, cc_ in late_w:
                            if cc_ == mc:
                                nc.vector.tensor_copy(out=w8[:, oc, xc],
                                                      in_=t[:])
                        # running denominator, query cols 0:1024 (bf16 DVE)
                        src = pT8[:, pair, par, 0:1024]
                        if mc == 0:
                            nc.vector.tensor_copy(out=dacc[:], in_=src)
                        else:
                            nc.vector.tensor_add(dacc[:], dacc[:], src)
                        if par == 1:
                            # pair complete: PE-side denominator for query
                            # cols 1024:2048 and the (2,0) ctx tile; pair 7
                            # closes each accumulation group
                            for q, p0 in ((0, DS2_P0), (1, DS3_P0)):
                                if pair >= p0:
                                    qc = slice(1024 + q * NB,
                                               1024 + (q + 1) * NB)
                                    nc.tensor.matmul(
                                        ds23[q][:], ones8[:],
                                        pT8[:, pair, :, qc],
                                        start=(pair == p0),
                                        stop=(pair == NPAIR - 1),
                                        perf_mode=DR)
                            if pair >= CTX20_P0:
                                nc.tensor.matmul(
                                    ctx20[:], vT8[:, pair, :, 0:P],
                                    pT8[:, pair, :, 1024:1536],
                                    start=(pair == CTX20_P0),
                                    stop=(pair == NPAIR - 1),
                                    perf_mode=DR)


                def ct_evict(ct, qt, cc, nsub, qpick, add_on_dve=False):
                    rows = slice(cc * P, (cc + 1) * P)
                    sub = NB // nsub
                    for si in range(nsub):
                        c0 = qt * NB + si * sub
                        cols = slice(c0, c0 + sub)
                        pcols = slice(si * sub, (si + 1) * sub)
                        t = evict.tile([P, sub], F32, tag="t", name="t")
                        nc.vector.tensor_mul(t[:], ct[:, pcols],
                                             recip[:, cols])
                        o = evict.tile([P, sub], BF16, tag="o", name="o")
                        eng = nc.vector if add_on_dve else nc.gpsimd
                        eng.tensor_add(o[:], t[:], xr[cc][:, cols])
                        deng = (nc.sync, nc.scalar,
                                nc.gpsimd)[(qpick + si) % 3]
                        deng.dma_start(out=out_d[rows, cols], in_=o[:])

                # ---- phase 3 head (inside psCL) ----
                # DVE order matters: recip2 then ctx20's recip-multiply
                # FIRST — that mul is the last reader of ctx20's PSUM bank,
                # which the first psDR tiles may land on (WAR)
                nc.vector.reciprocal_approx_fast(out=recip[:, 1024:1536],
                                                 in_=ds23[0][:])
                t20 = evict.tile([P, NB], F32, tag="t", name="t20")
                nc.vector.tensor_mul(t20[:], ctx20[:], recip[:, 1024:1536])
                nc.vector.reciprocal_approx_fast(out=recip[:, 1536:2048],
                                                 in_=ds23[1][:])
                # residual prep, pinned behind the denominator so the
                # scheduler cannot hoist it into the scores-phase DVE queue
                nc.vector.tensor_scalar(out=bv_late[:], in0=bv_sb[:],
                                        scalar1=ds23[0][:, 0:1],
                                        scalar2=ds23[0][:, 0:1],
                                        op0=mybir.AluOpType.add,
                                        op1=mybir.AluOpType.subtract)
                for cc in range(2):
                    nc.vector.tensor_scalar_add(out=xr[cc][:],
                                                in0=xbf[cc][:],
                                                scalar1=bv_late[:, cc, :])
                o20 = evict.tile([P, NB], BF16, tag="o", name="o20")
                nc.gpsimd.tensor_add(o20[:], t20[:], xr[0][:, 1024:1536])
                nc.sync.dma_start(out=out_d[0:P, 1024:1536], in_=o20[:])

            # ---- phase 3: denom finish + ctx-right + epilogue ----
            with tc.tile_pool(name="psDR", bufs=1,
                              space=bass.MemorySpace.PSUM) as psDR:
                def ctx_mms(ct, qt, cc):
                    for pr in range(NPAIR):
                        nc.tensor.matmul(
                            ct[:],
                            vT8[:, pr, :, cc * P:(cc + 1) * P],
                            pT8[:, pr, :, qt * NB:(qt + 1) * NB],
                            start=(pr == 0), stop=(pr == NPAIR - 1),
                            perf_mode=DR)

                # finish the denominator: qt0/qt1 from the bf16 accumulator
                ds01 = []
                for q in range(2):
                    cols = slice(q * NB, (q + 1) * NB)
                    ds = psDR.tile([P, NB], F32, tag="ds", name="ds",
                                   bufs=2)
                    ds01.append(ds)
                    nc.tensor.matmul(ds[:], ones_bf[:, 0:P],
                                     dacc[:, cols],
                                     start=True, stop=True)
                nc.vector.reciprocal_approx_fast(out=recip[:, 0:512],
                                                 in_=ds01[0][:])
                nc.vector.reciprocal_approx_fast(out=recip[:, 512:1024],
                                                 in_=ds01[1][:])
                ctxR = {}
                order = ((2, 1), (3, 0), (3, 1), (0, 0), (0, 1),
                         (1, 0), (1, 1))
                evict_after = {
                    (2, 1): [],
                    (3, 0): [((2, 1), 1, 1, False)],
                    (3, 1): [((3, 0), 1, 2, False)],
                    (0, 0): [((3, 1), 1, 0, False)],
                    (0, 1): [((0, 0), 1, 1, False)],
                    (1, 0): [((0, 1), 1, 2, False)],
                    (1, 1): [((1, 0), 2, 0, True)],
                }
                all_tiles = {}
                for qt, cc in order:
                    ct = psDR.tile([P, NB], F32, tag="cr", name="cr",
                                   bufs=6)
                    ctxR[(qt, cc)] = ct
                    all_tiles[(qt, cc)] = ct
                    ctx_mms(ct, qt, cc)
                    for (eqt, ecc), nsub, qpick, dve in evict_after[(qt, cc)]:
                        ct_evict(all_tiles[(eqt, ecc)], eqt, ecc, nsub,
                                 qpick, add_on_dve=dve)
                ct_evict(ctxR[(1, 1)], 1, 1, 4, 2, add_on_dve=True)

    nc.compile()
    return nc


def get_compiled():
    global _COMPILED
    if _COMPILED is None:
        _COMPILED = build_nc()
    return _COMPILED


def make_in_maps(inputs):
    f8 = ml_dtypes.float8_e4m3
    x = np.ascontiguousarray(np.asarray(inputs["x"], dtype=np.float32))
    Wq = np.asarray(inputs["Wq"], np.float32)
    Wk = np.asarray(inputs["Wk"], np.float32)
    Wv = np.asarray(inputs["Wv"], np.float32)
    bq = np.asarray(inputs["bq"], np.float32)
    M = Wq.T @ Wk                               # scores_raw = x^T M x
    u = SCALE * (Wk.T @ bq)                     # per-key score bias u.x
    wvu = np.zeros((C, 272), np.float32)
    wvu[:, 0:C] = Wv.T
    wvu[:, C] = u
    shared = {
        "mt8": np.ascontiguousarray(KAPPA * M.T).astype(f8),
        "wvu8": wvu.astype(f8),
        "bv": np.asarray(inputs["bv"], np.float32).reshape(C, 1),
    }
    return [{"x8": x[i].astype(f8), "xbf": x[i].astype(ml_dtypes.bfloat16),
             **shared} for i in range(B)]


def run(inputs, trace=False, **kwargs):
    nc = get_compiled()
    res = run_bass_kernel_spmd(nc, make_in_maps(inputs),
                               core_ids=list(range(B)), trace=trace, **kwargs)
    out = np.stack([res.results[i]["out"] for i in range(B)], axis=0)
    return out.astype(np.float32), res


def kernel(**inputs):
    out, _ = run(inputs)
    return out


# revision 26
# speedup vs baseline: 1.0088x; 1.0088x over previous
"""Trainium2 Bass kernel for AttentionBlock (B=8, C=256, L=2048), data-parallel
over batch across 8 NeuronCores.

Math (one batch per core, x: [C, L]):
    t^T   = w8^T x8            w8 = fp8(kappa M x),  M = Wq^T Wk,  kappa = 128*SCALE/ln2
    pT    = exp(t*ln2/128 + ux)  [m, l], keys m on partitions, fp8 direct from ACT
    denom = per-query sum of pT: cols 0:1024 via bf16 DVE accumulator + ones
            matmul, cols 1024:2048 via fp8 DoubleRow ones-matmuls in PSUM
    ctx   = vT^T pT in fp8 DoubleRow over chunk PAIRS (256 keys/instruction)
    out   = ctx * (1/denom) + (bf16(x) + bv)

Schedule (v10):
  - exp: 2 x 1024-wide ACT instructions per chunk writing fp8 pT directly;
    ACT paces phase 2 at ~2.26us/chunk
  - x8 loads ride FIVE dma queues (sync/scalar/gpsimd/vector/tensor) in
    256-col slices so all of x8 is resident by ~11.5us; xbf rides last on
    the scalar queue (coarse per-queue completion semaphores would
    otherwise stall x8 consumers behind the 1MB xbf transfer)
  - no separate w-projection PSUM pool: cols 0:1024 go through two score
    tile rotations before chunk 0; cols 1024:2048 borrow the ds2/ds3/ctx20
    banks as scratch during chunks 1-4 (their accumulations start at pairs
    3/2/1 and the missed pairs are caught up at the start of phase 3)
  - PSUM: scores 2x[P,1024] (4) + vp (1) + ds2/ds3 (2) + ctx(2,0) (1)
  - phase 3: catch-up mms, denominator finish, 7 remaining ctx tiles in
    fp8 DR with evictions interleaved; tail tiles' residual adds on DVE
    (gpsimd tensor_add is ~1.1us/512) and the last tile evicts in 4
    sub-slices across 3 dma queues
"""

import math
import numpy as np
import ml_dtypes

import concourse.bass as bass
import concourse.tile as tile
from concourse import bacc, mybir
from concourse.bass_utils import run_bass_kernel_spmd

B, C, L = 8, 256, 2048
P = 128                 # partitions
NMC = L // P            # 16 m-chunks (key blocks)
NPAIR = NMC // 2        # 8 key pairs (256 keys each)
NB = 512                # matmul moving free dim
SCALE = float(C) ** -0.5
LN2 = math.log(2.0)
KAPPA = 128.0 * SCALE / LN2     # scores t = kappa * s_raw (baked into mt8 on host)

F32 = mybir.dt.float32
BF16 = mybir.dt.bfloat16
F8 = mybir.dt.float8e4
DR = mybir.MatmulPerfMode.DoubleRow

# first phase-2 pair each accumulator covers (earlier pairs caught up in
# phase 3); constrained by when each tile is free of w-projection scratch
CTX20_P0, DS2_P0, DS3_P0 = 1, 3, 2

_COMPILED = None


def build_nc():
    nc = bacc.Bacc("TRN2", target_bir_lowering=False, debug=False, num_devices=8)

    x8_d = nc.dram_tensor("x8", [C, L], F8, kind="ExternalInput").ap()
    xbf_d = nc.dram_tensor("xbf", [C, L], BF16, kind="ExternalInput").ap()
    mt8_d = nc.dram_tensor("mt8", [C, C], F8, kind="ExternalInput").ap()
    wvu8_d = nc.dram_tensor("wvu8", [C, 272], F8, kind="ExternalInput").ap()
    bv_d = nc.dram_tensor("bv", [C, 1], F32, kind="ExternalInput").ap()
    out_d = nc.dram_tensor("out", [C, L], BF16, kind="ExternalOutput").ap()

    with tile.TileContext(nc) as tc:
        with (
            tc.tile_pool(name="const", bufs=1) as const,
            tc.tile_pool(name="data", bufs=1) as data,
            tc.tile_pool(name="evict", bufs=6) as evict,
        ):
            # ---- constants ----
            ones_bf = const.tile([P, NB], BF16)
            nc.vector.memset(ones_bf[:], 1.0)
            ones8 = const.tile([P, 2, P], F8)
            nc.gpsimd.memset(ones8[:], 1.0)
            tiny = const.tile([P, 4, 16], F32)

            x8 = data.tile([P, 2, L], F8, tag="x8", name="x8")
            xbf = [data.tile([P, L], BF16, tag=f"xbf{c}", name=f"xbf{c}")
                   for c in range(2)]
            mt8 = const.tile([P, 2, C], F8, tag="mt8")
            wvu8 = const.tile([P, 2, 272], F8, tag="wvu8")
            bv_sb = const.tile([P, 2, 1], F32, tag="bv")

            def x8_dma(c0, c1, eng):
                cols = slice(c0, c1)
                eng.dma_start(out=x8[:, :, cols],
                              in_=x8_d[:, cols].rearrange("(j p) l -> p j l",
                                                          p=P))

            # x8 in 256-col slices across the 3 dma-capable queues in
            # consumption order; weights early on scalar. bv/xbf descriptor
            # generation is deferred (traced after the w8 evicts) so it
            # doesn't occupy the ACT/sync engines during startup, and xbf
            # rides last on the scalar queue (coarse per-queue completion
            # semaphores would otherwise stall x8 consumers behind it).
            nc.scalar.dma_start(out=mt8[:],
                                in_=mt8_d.rearrange("(j p) o -> p j o", p=P))
            x8_dma(0, 256, nc.gpsimd)
            x8_dma(256, 512, nc.sync)
            nc.scalar.dma_start(out=wvu8[:],
                                in_=wvu8_d.rearrange("(j p) o -> p j o", p=P))
            x8_dma(512, 768, nc.gpsimd)
            x8_dma(768, 1024, nc.sync)
            x8_dma(1024, 1280, nc.scalar)
            x8_dma(1280, 1536, nc.gpsimd)
            x8_dma(1536, 1792, nc.sync)
            x8_dma(1792, 2048, nc.gpsimd)

            w8 = data.tile([P, 2, L], F8, tag="w8", name="w8")
            vT8 = data.tile([P, NPAIR, 2, C], F8, tag="vT8")
            pT8 = data.tile([P, NPAIR, 2, L], F8, tag="pT8")
            b_act = data.tile([P, NMC, 1], F32, tag="b_act")
            bv_late = data.tile([P, 2, 1], F32, tag="bv_late")
            dacc = data.tile([P, 1024], BF16, tag="dacc")
            recip = data.tile([P, L], F32, tag="recip")
            xr = [data.tile([P, L], BF16, tag=f"xr{c}", name=f"xr{c}")
                  for c in range(2)]

            with tc.tile_pool(name="psCL", bufs=1,
                              space=bass.MemorySpace.PSUM) as psCL:
                ctx20 = psCL.tile([P, NB], F32, tag="c20", name="c20", bufs=1)
                ds23 = [psCL.tile([P, NB], F32, tag=f"ds{q}", name=f"ds{q}",
                                  bufs=1) for q in (2, 3)]

                with tc.tile_pool(name="psB", bufs=1,
                                  space=bass.MemorySpace.PSUM) as psB:
                    # ---- warmup: ACT/DVE table loads + PE spin-up; matmul
                    # targets are overwritten later via start=True ----
                    nc.vector.memset(tiny[:, 0, :], 1.0)
                    nc.scalar.activation(out=tiny[:, 1, :], in_=tiny[:, 0, :],
                                         func=mybir.ActivationFunctionType.Exp,
                                         scale=1.0)
                    nc.vector.reciprocal_approx_fast(out=tiny[:, 2, :],
                                                     in_=tiny[:, 0, :])
                    for i in range(3):
                        nc.tensor.matmul((ds23 + [ctx20])[i][:],
                                         ones_bf[:, 0:P], ones_bf[:],
                                         start=True, stop=True)
                    nc.tensor.matmul(ds23[0][0:32, 0:16], ones8[:, :, 0:32],
                                     ones8[:, :, 0:16], start=True, stop=True,
                                     perf_mode=DR)

                    # ---- w projection cols 0:1024 through two score-tile
                    # rotations (w = kappa M x, DoubleRow, 256 deep) ----
                    wpA = [psB.tile([P, 1024], F32, tag="s", name=f"wpA{oc}",
                                    bufs=2) for oc in range(2)]

                    def wmm(dst, dcols, oc, xcols):
                        nc.tensor.matmul(dst[:, dcols],
                                         mt8[:, :, oc * P:(oc + 1) * P],
                                         x8[:, :, xcols],
                                         start=True, stop=True, perf_mode=DR)

                    # evicts trace right after their block's matmuls: the
                    # completion semaphores are coarse (threshold = all PE
                    # work traced so far), so an evict traced after ALL four
                    # matmuls waits for all four. ACT takes oc0, DVE oc1.
                    for b in range(2):
                        cols = slice(b * NB, (b + 1) * NB)
                        for oc in range(2):
                            wmm(wpA[oc], cols, oc, cols)
                        nc.scalar.copy(out=w8[:, 0, cols], in_=wpA[0][:, cols])
                        nc.vector.tensor_copy(out=w8[:, 1, cols],
                                              in_=wpA[1][:, cols])
                    # deferred descriptor generation (data needed in phase 3)
                    nc.scalar.dma_start(
                        out=bv_sb[:],
                        in_=bv_d.rearrange("(j p) o -> p j o", p=P))
                    nc.scalar.dma_start(out=xbf[0][:], in_=xbf_d[0:P, :])
                    nc.scalar.dma_start(out=xbf[1][:], in_=xbf_d[P:C, :])

                    # late w-projection (cols 1024:2048) borrows psCL banks as
                    # scratch: (tile, psum cols, w8 oc, x8 cols, mm chunk,
                    # cast chunk)
                    late_w = [
                        (ds23[0], 0, slice(1024, 1536), 1, 1),
                        (ds23[1], 1, slice(1024, 1536), 1, 2),
                        (ctx20, 0, slice(1536, 2048), 2, 3),
                        (ds23[0], 1, slice(1536, 2048), 4, 5),
                    ]

                    for mc in range(NMC):
                        pair, par = mc // 2, mc % 2
                        mrows = slice(mc * P, (mc + 1) * P)
                        # v/ux projection for this key chunk
                        vp = psB.tile([P, 272], F32, tag="vp", name="vp",
                                      bufs=1)
                        nc.tensor.matmul(
                            vp[:], x8[:, :, mrows], wvu8[:],
                            start=True, stop=True, perf_mode=DR)
                        # scores, two 1024-wide tiles (2 x 512 mms each)
                        s = [psB.tile([P, 1024], F32, tag="s", name="s",
                                      bufs=2) for _ in range(2)]
                        for h in range(2):
                            for ln in range(2):
                                q0 = h * 1024 + ln * NB
                                nc.tensor.matmul(
                                    s[h][:, ln * NB:(ln + 1) * NB],
                                    w8[:, :, mrows],
                                    x8[:, :, q0:q0 + NB],
                                    start=True, stop=True, perf_mode=DR)
                        # late w-projection matmuls scheduled for this chunk
                        for t, oc, xc, mmc, cc_ in late_w:
                            if mmc == mc:
                                wmm(t, slice(0, NB), oc, xc)
                        # catch-up mms for the pairs the accumulators missed
                        # while holding w-projection scratch. Traced at the
                        # top of chunk 15, before its exps: they fill the PE
                        # idle window while pair-7 work waits on the final
                        # exps — a >1us PE stall here would drop the PE clock
                        # to its mid p-state for the next ~3us of phase 3.
                        if mc == 15:
                            for q, prs in ((0, (0, 1, 2)), (1, (0, 1))):
                                qc = slice(1024 + q * NB,
                                           1024 + (q + 1) * NB)
                                for pr in prs:
                                    nc.tensor.matmul(ds23[q][:], ones8[:],
                                                     pT8[:, pr, :, qc],
                                                     start=False, stop=False,
                                                     perf_mode=DR)
                            nc.tensor.matmul(ctx20[:], vT8[:, 0, :, 0:P],
                                             pT8[:, 0, :, 1024:1536],
                                             start=False, stop=False,
                                             perf_mode=DR)
                        # per-key exp bias (ux); vT frees the vp bank
                        nc.vector.tensor_copy(out=b_act[:, mc, :],
                                              in_=vp[:, C:C + 1])
                        nc.vector.tensor_copy(out=vT8[:, pair, par, :],
                                              in_=vp[:, 0:C])
                        # exp -> fp8 pT, 1024 cols per instruction
                        for h in range(2):
                            nc.scalar.activation(
                                out=pT8[:, pair, par, h * 1024:(h + 1) * 1024],
                                in_=s[h][:],
                                func=mybir.ActivationFunctionType.Exp,
                                scale=LN2 / 128.0, bias=b_act[:, mc, :])
                        # late w-projection evictions (DVE, one per chunk)
                        for t, oc, xc, mmc, cc_ in late_w:
                            if cc_ == mc:
                                nc.vector.tensor_copy(out=w8[:, oc, xc],
                                                      in_=t[:])
                        # running denominator, query cols 0:1024 (bf16 DVE)
                        src = pT8[:, pair, par, 0:1024]
                        if mc == 0:
                            nc.vector.tensor_copy(out=dacc[:], in_=src)
                        else:
                            nc.vector.tensor_add(dacc[:], dacc[:], src)
                        if par == 1:
                            # pair complete: PE-side denominator for query
                            # cols 1024:2048 and the (2,0) ctx tile; pair 7
                            # closes each accumulation group
                            for q, p0 in ((0, DS2_P0), (1, DS3_P0)):
                                if pair >= p0:
                                    qc = slice(1024 + q * NB,
                                               1024 + (q + 1) * NB)
                                    nc.tensor.matmul(
                                        ds23[q][:], ones8[:],
                                        pT8[:, pair, :, qc],
                                        start=(pair == p0),
                                        stop=(pair == NPAIR - 1),
                                        perf_mode=DR)
                            if pair >= CTX20_P0:
                                nc.tensor.matmul(
                                    ctx20[:], vT8[:, pair, :, 0:P],
                                    pT8[:, pair, :, 1024:1536],
                                    start=(pair == CTX20_P0),
                                    stop=(pair == NPAIR - 1),
                                    perf_mode=DR)


                def ct_evict(ct, qt, cc, nsub, qpick, add_on_dve=False):
                    rows = slice(cc * P, (cc + 1) * P)
                    sub = NB // nsub
                    for si in range(nsub):
                        c0 = qt * NB + si * sub
                        cols = slice(c0, c0 + sub)
                        pcols = slice(si * sub, (si + 1) * sub)
                        t = evict.tile([P, sub], F32, tag="t", name="t")
                        nc.vector.tensor_mul(t[:], ct[:, pcols],
                                             recip[:, cols])
                        o = evict.tile([P, sub], BF16, tag="o", name="o")
                        eng = nc.vector if add_on_dve else nc.gpsimd
                        eng.tensor_add(o[:], t[:], xr[cc][:, cols])
                        deng = (nc.sync, nc.scalar,
                                nc.gpsimd)[(qpick + si) % 3]
                        deng.dma_start(out=out_d[rows, cols], in_=o[:])

                def ctx_mms(ct, qt, cc):
                    for pr in range(NPAIR):
                        nc.tensor.matmul(
                            ct[:],
                            vT8[:, pr, :, cc * P:(cc + 1) * P],
                            pT8[:, pr, :, qt * NB:(qt + 1) * NB],
                            start=(pr == 0), stop=(pr == NPAIR - 1),
                            perf_mode=DR)

                # ---- phase 3a (psCL still open, 5 banks): the first ctx
                # tiles start while the final exps run — pairs 0-6 have no
                # dependence on chunk 15 and keep the PE clock from dropping
                with tc.tile_pool(name="psD1", bufs=1,
                                  space=bass.MemorySpace.PSUM) as psD1:
                    ctx21 = psD1.tile([P, NB], F32, tag="cr1", name="cr21",
                                      bufs=1)
                    ctx_mms(ctx21, 2, 1)
                    # DVE order matters: recip2 then ctx20's recip-multiply
                    # FIRST — it is the last reader of ctx20's PSUM bank
                    nc.vector.reciprocal_approx_fast(out=recip[:, 1024:1536],
                                                     in_=ds23[0][:])
                    t20 = evict.tile([P, NB], F32, tag="t", name="t20")
                    nc.vector.tensor_mul(t20[:], ctx20[:],
                                         recip[:, 1024:1536])
                    nc.vector.reciprocal_approx_fast(out=recip[:, 1536:2048],
                                                     in_=ds23[1][:])
                    # residual prep, pinned behind the denominator so the
                    # scheduler cannot hoist it into the scores-phase queue
                    nc.vector.tensor_scalar(out=bv_late[:], in0=bv_sb[:],
                                            scalar1=ds23[0][:, 0:1],
                                            scalar2=ds23[0][:, 0:1],
                                            op0=mybir.AluOpType.add,
                                            op1=mybir.AluOpType.subtract)
                    for cc in range(2):
                        nc.vector.tensor_scalar_add(out=xr[cc][:],
                                                    in0=xbf[cc][:],
                                                    scalar1=bv_late[:, cc, :])
                    o20 = evict.tile([P, NB], BF16, tag="o", name="o20")
                    nc.gpsimd.tensor_add(o20[:], t20[:], xr[0][:, 1024:1536])
                    nc.sync.dma_start(out=out_d[0:P, 1024:1536], in_=o20[:])

                    # denominator for qt0/qt1 from the bf16 accumulator
                    ds01 = []
                    for q in range(2):
                        cols = slice(q * NB, (q + 1) * NB)
                        ds = psD1.tile([P, NB], F32, tag="ds", name="ds",
                                       bufs=2)
                        ds01.append(ds)
                        nc.tensor.matmul(ds[:], ones_bf[:, 0:P],
                                         dacc[:, cols],
                                         start=True, stop=True)
                    nc.vector.reciprocal_approx_fast(out=recip[:, 0:512],
                                                     in_=ds01[0][:])
                    nc.vector.reciprocal_approx_fast(out=recip[:, 512:1024],
                                                     in_=ds01[1][:])
                    ctx30 = psD1.tile([P, NB], F32, tag="cr2", name="cr30",
                                      bufs=1)
                    ctx_mms(ctx30, 3, 0)
                    ct_evict(ctx21, 2, 1, 1, 1)
                    ctx31 = psD1.tile([P, NB], F32, tag="cr1", name="cr31",
                                      bufs=1)
                    ctx_mms(ctx31, 3, 1)
                    ct_evict(ctx30, 3, 0, 1, 2)
                    # (3,1)'s eviction must trace before psD1 closes so
                    # psD2's tiles inherit the WAR edge on its bank
                    ct_evict(ctx31, 3, 1, 1, 0)

            # ---- phase 3b: remaining ctx tiles + epilogue ----
            with tc.tile_pool(name="psD2", bufs=1,
                              space=bass.MemorySpace.PSUM) as psD2:
                order = ((0, 0), (0, 1), (1, 0), (1, 1))
                evict_after = {
                    (0, 0): [],
                    (0, 1): [((0, 0), 1, 1, False)],
                    (1, 0): [((0, 1), 1, 2, False)],
                    (1, 1): [((1, 0), 2, 0, True)],
                }
                all_tiles = {}
                ctxR = {}
                for qt, cc in order:
                    ct = psD2.tile([P, NB], F32, tag="cr", name="cr",
                                   bufs=4)
                    ctxR[(qt, cc)] = ct
                    all_tiles[(qt, cc)] = ct
                    ctx_mms(ct, qt, cc)
                    for (eqt, ecc), nsub, qpick, dve in evict_after[(qt, cc)]:
                        ct_evict(all_tiles[(eqt, ecc)], eqt, ecc, nsub,
                                 qpick, add_on_dve=dve)
                ct_evict(ctxR[(1, 1)], 1, 1, 4, 2, add_on_dve=True)

    nc.compile()
    return nc


def get_compiled():
    global _COMPILED
    if _COMPILED is None:
        _COMPILED = build_nc()
    return _COMPILED


def make_in_maps(inputs):
    f8 = ml_dtypes.float8_e4m3
    x = np.ascontiguousarray(np.asarray(inputs["x"], dtype=np.float32))
    Wq = np.asarray(inputs["Wq"], np.float32)
    Wk = np.asarray(inputs["Wk"], np.float32)
    Wv = np.asarray(inputs["Wv"], np.float32)
    bq = np.asarray(inputs["bq"], np.float32)
    M = Wq.T @ Wk                               # scores_raw = x^T M x
    u = SCALE * (Wk.T @ bq)                     # per-key score bias u.x
    wvu = np.zeros((C, 272), np.float32)
    wvu[:, 0:C] = Wv.T
    wvu[:, C] = u
    shared = {
        "mt8": np.ascontiguousarray(KAPPA * M.T).astype(f8),
        "wvu8": wvu.astype(f8),
        "bv": np.asarray(inputs["bv"], np.float32).reshape(C, 1),
    }
    return [{"x8": x[i].astype(f8), "xbf": x[i].astype(ml_dtypes.bfloat16),
             **shared} for i in range(B)]


def run(inputs, trace=False, **kwargs):
    nc = get_compiled()
    res = run_bass_kernel_spmd(nc, make_in_maps(inputs),
                               core_ids=list(range(B)), trace=trace, **kwargs)
    out = np.stack([res.results[i]["out"] for i in range(B)], axis=0)
    return out.astype(np.float32), res


def kernel(**inputs):
    out, _ = run(inputs)
    return out


# revision 27
# speedup vs baseline: 1.0536x; 1.0444x over previous
"""Trainium2 Bass kernel for AttentionBlock (B=8, C=256, L=2048), data-parallel
over batch across 8 NeuronCores.

Math (one batch per core, x: [C, L]):
    t^T   = w8^T x8            w8 = fp8(kappa M x),  M = Wq^T Wk,  kappa = 128*SCALE/ln2
    pT    = exp(t*ln2/128 + ux)  [m, l], keys m on partitions, fp8 direct from ACT
    denom = per-query sum of pT: cols 0:1024 via bf16 DVE accumulator + ones
            matmul, cols 1024:2048 via fp8 DoubleRow ones-matmuls in PSUM
    ctx   = vT^T pT in fp8 DoubleRow over chunk PAIRS (256 keys/instruction)
    out   = ctx * (1/denom) + (bf16(x) + bv)

Schedule (v10):
  - exp: 2 x 1024-wide ACT instructions per chunk writing fp8 pT directly;
    ACT paces phase 2 at ~2.26us/chunk
  - x8 loads ride FIVE dma queues (sync/scalar/gpsimd/vector/tensor) in
    256-col slices so all of x8 is resident by ~11.5us; xbf rides last on
    the scalar queue (coarse per-queue completion semaphores would
    otherwise stall x8 consumers behind the 1MB xbf transfer)
  - no separate w-projection PSUM pool: cols 0:1024 go through two score
    tile rotations before chunk 0; cols 1024:2048 borrow the ds2/ds3/ctx20
    banks as scratch during chunks 1-4 (their accumulations start at pairs
    3/2/1 and the missed pairs are caught up at the start of phase 3)
  - PSUM: scores 2x[P,1024] (4) + vp (1) + ds2/ds3 (2) + ctx(2,0) (1)
  - phase 3: catch-up mms, denominator finish, 7 remaining ctx tiles in
    fp8 DR with evictions interleaved; tail tiles' residual adds on DVE
    (gpsimd tensor_add is ~1.1us/512) and the last tile evicts in 4
    sub-slices across 3 dma queues
"""

import math
import numpy as np
import ml_dtypes

import concourse.bass as bass
import concourse.tile as tile
from concourse import bacc, mybir
from concourse.bass_utils import run_bass_kernel_spmd

B, C, L = 8, 256, 2048
P = 128                 # partitions
NMC = L // P            # 16 m-chunks (key blocks)
NPAIR = NMC // 2        # 8 key pairs (256 keys each)
NB = 512                # matmul moving free dim
SCALE = float(C) ** -0.5
LN2 = math.log(2.0)
KAPPA = 128.0 * SCALE / LN2     # scores t = kappa * s_raw (baked into mt8 on host)

F32 = mybir.dt.float32
BF16 = mybir.dt.bfloat16
F8 = mybir.dt.float8e4
DR = mybir.MatmulPerfMode.DoubleRow

# first phase-2 pair each accumulator covers (earlier pairs caught up in
# phase 3); constrained by when each tile is free of w-projection scratch
CTX20_P0, DS2_P0, DS3_P0 = 1, 3, 2

_COMPILED = None


def build_nc():
    nc = bacc.Bacc("TRN2", target_bir_lowering=False, debug=False, num_devices=8)

    x8_d = nc.dram_tensor("x8", [C, L], F8, kind="ExternalInput").ap()
    xbf_d = nc.dram_tensor("xbf", [C, L], BF16, kind="ExternalInput").ap()
    mt8_d = nc.dram_tensor("mt8", [C, C], F8, kind="ExternalInput").ap()
    wvu8_d = nc.dram_tensor("wvu8", [C, 272], F8, kind="ExternalInput").ap()
    bv_d = nc.dram_tensor("bv", [C, 1], F32, kind="ExternalInput").ap()
    out_d = nc.dram_tensor("out", [C, L], BF16, kind="ExternalOutput").ap()

    with tile.TileContext(nc) as tc:
        with (
            tc.tile_pool(name="const", bufs=1) as const,
            tc.tile_pool(name="data", bufs=1) as data,
            tc.tile_pool(name="evict", bufs=6) as evict,
        ):
            # ---- constants ----
            ones_bf = const.tile([P, NB], BF16)
            nc.vector.memset(ones_bf[:], 1.0)
            ones8 = const.tile([P, 2, P], F8)
            nc.gpsimd.memset(ones8[:], 1.0)
            tiny = const.tile([P, 4, 16], F32)

            x8 = data.tile([P, 2, L], F8, tag="x8", name="x8")
            xbf = [data.tile([P, L], BF16, tag=f"xbf{c}", name=f"xbf{c}")
                   for c in range(2)]
            mt8 = const.tile([P, 2, C], F8, tag="mt8")
            wvu8 = const.tile([P, 2, 272], F8, tag="wvu8")
            bv_sb = const.tile([P, 2, 1], F32, tag="bv")

            def x8_dma(j, c0, c1, eng):
                # per-j slices keep DRAM reads row-contiguous: [128 rows,
                # c1-c0 bytes] per transfer. Column-slicing BOTH j halves
                # would cut packets to (c1-c0)/2... no — j-split is what
                # keeps each descriptor a single contiguous DRAM run.
                eng.dma_start(out=x8[:, j, c0:c1],
                              in_=x8_d[j * P:(j + 1) * P, c0:c1])

            # x8 loads: a narrow column head (cols 0:512, 512B packets) lets
            # the w projection start early; the remaining cols 512:2048 move
            # as two row-major 192KB transfers (1.5KB packets, ~2x the
            # per-queue rate of column slices). Weights first on scalar.
            # bv/xbf descriptor generation is deferred (traced after the w8
            # evicts) and xbf rides last on the scalar queue: the per-queue
            # completion semaphores are coarse, so x8 consumers would stall
            # behind the 1MB xbf transfer otherwise.
            nc.scalar.dma_start(out=mt8[:],
                                in_=mt8_d.rearrange("(j p) o -> p j o", p=P))
            x8_dma(0, 0, 512, nc.gpsimd)
            x8_dma(1, 0, 512, nc.sync)
            nc.scalar.dma_start(out=wvu8[:],
                                in_=wvu8_d.rearrange("(j p) o -> p j o", p=P))
            x8_dma(0, 512, 2048, nc.gpsimd)
            x8_dma(1, 512, 2048, nc.sync)

            w8 = data.tile([P, 2, L], F8, tag="w8", name="w8")
            vT8 = data.tile([P, NPAIR, 2, C], F8, tag="vT8")
            pT8 = data.tile([P, NPAIR, 2, L], F8, tag="pT8")
            b_act = data.tile([P, NMC, 1], F32, tag="b_act")
            bv_late = data.tile([P, 2, 1], F32, tag="bv_late")
            dacc = data.tile([P, 1024], BF16, tag="dacc")
            recip = data.tile([P, L], F32, tag="recip")
            xr = [data.tile([P, L], BF16, tag=f"xr{c}", name=f"xr{c}")
                  for c in range(2)]

            with tc.tile_pool(name="psCL", bufs=1,
                              space=bass.MemorySpace.PSUM) as psCL:
                ctx20 = psCL.tile([P, NB], F32, tag="c20", name="c20", bufs=1)
                ds23 = [psCL.tile([P, NB], F32, tag=f"ds{q}", name=f"ds{q}",
                                  bufs=1) for q in (2, 3)]

                with tc.tile_pool(name="psB", bufs=1,
                                  space=bass.MemorySpace.PSUM) as psB:
                    # ---- warmup: ACT/DVE table loads + PE spin-up; matmul
                    # targets are overwritten later via start=True ----
                    nc.vector.memset(tiny[:, 0, :], 1.0)
                    nc.scalar.activation(out=tiny[:, 1, :], in_=tiny[:, 0, :],
                                         func=mybir.ActivationFunctionType.Exp,
                                         scale=1.0)
                    nc.vector.reciprocal_approx_fast(out=tiny[:, 2, :],
                                                     in_=tiny[:, 0, :])
                    for i in range(3):
                        nc.tensor.matmul((ds23 + [ctx20])[i][:],
                                         ones_bf[:, 0:P], ones_bf[:],
                                         start=True, stop=True)
                    nc.tensor.matmul(ds23[0][0:32, 0:16], ones8[:, :, 0:32],
                                     ones8[:, :, 0:16], start=True, stop=True,
                                     perf_mode=DR)

                    # ---- w projection cols 0:1024 through two score-tile
                    # rotations (w = kappa M x, DoubleRow, 256 deep) ----
                    wpA = [psB.tile([P, 1024], F32, tag="s", name=f"wpA{oc}",
                                    bufs=2) for oc in range(2)]

                    def wmm(dst, dcols, oc, xcols):
                        nc.tensor.matmul(dst[:, dcols],
                                         mt8[:, :, oc * P:(oc + 1) * P],
                                         x8[:, :, xcols],
                                         start=True, stop=True, perf_mode=DR)

                    # evicts trace right after their block's matmuls: the
                    # completion semaphores are coarse (threshold = all PE
                    # work traced so far), so an evict traced after ALL four
                    # matmuls waits for all four. ACT takes oc0, DVE oc1.
                    for b in range(2):
                        cols = slice(b * NB, (b + 1) * NB)
                        for oc in range(2):
                            wmm(wpA[oc], cols, oc, cols)
                        nc.scalar.copy(out=w8[:, 0, cols], in_=wpA[0][:, cols])
                        nc.vector.tensor_copy(out=w8[:, 1, cols],
                                              in_=wpA[1][:, cols])
                    # deferred descriptor generation (data needed in phase 3)
                    nc.scalar.dma_start(
                        out=bv_sb[:],
                        in_=bv_d.rearrange("(j p) o -> p j o", p=P))
                    nc.scalar.dma_start(out=xbf[0][:], in_=xbf_d[0:P, :])
                    nc.scalar.dma_start(out=xbf[1][:], in_=xbf_d[P:C, :])

                    # late w-projection (cols 1024:2048) borrows psCL banks as
                    # scratch: (tile, psum cols, w8 oc, x8 cols, mm chunk,
                    # cast chunk)
                    late_w = [
                        (ds23[0], 0, slice(1024, 1536), 1, 1),
                        (ds23[1], 1, slice(1024, 1536), 1, 2),
                        (ctx20, 0, slice(1536, 2048), 2, 3),
                        (ds23[0], 1, slice(1536, 2048), 4, 5),
                    ]

                    for mc in range(NMC):
                        pair, par = mc // 2, mc % 2
                        mrows = slice(mc * P, (mc + 1) * P)
                        # v/ux projection for this key chunk
                        vp = psB.tile([P, 272], F32, tag="vp", name="vp",
                                      bufs=1)
                        nc.tensor.matmul(
                            vp[:], x8[:, :, mrows], wvu8[:],
                            start=True, stop=True, perf_mode=DR)
                        # scores, two 1024-wide tiles (2 x 512 mms each)
                        s = [psB.tile([P, 1024], F32, tag="s", name="s",
                                      bufs=2) for _ in range(2)]
                        for h in range(2):
                            for ln in range(2):
                                q0 = h * 1024 + ln * NB
                                nc.tensor.matmul(
                                    s[h][:, ln * NB:(ln + 1) * NB],
                                    w8[:, :, mrows],
                                    x8[:, :, q0:q0 + NB],
                                    start=True, stop=True, perf_mode=DR)
                        # late w-projection matmuls scheduled for this chunk
                        for t, oc, xc, mmc, cc_ in late_w:
                            if mmc == mc:
                                wmm(t, slice(0, NB), oc, xc)
                        # catch-up mms for the pairs the accumulators missed
                        # while holding w-projection scratch. Traced at the
                        # top of chunk 15, before its exps: they fill the PE
                        # idle window while pair-7 work waits on the final
                        # exps — a >1us PE stall here would drop the PE clock
                        # to its mid p-state for the next ~3us of phase 3.
                        if mc == 15:
                            for q, prs in ((0, (0, 1, 2)), (1, (0, 1))):
                                qc = slice(1024 + q * NB,
                                           1024 + (q + 1) * NB)
                                for pr in prs:
                                    nc.tensor.matmul(ds23[q][:], ones8[:],
                                                     pT8[:, pr, :, qc],
                                                     start=False, stop=False,
                                                     perf_mode=DR)
                            nc.tensor.matmul(ctx20[:], vT8[:, 0, :, 0:P],
                                             pT8[:, 0, :, 1024:1536],
                                             start=False, stop=False,
                                             perf_mode=DR)
                        # per-key exp bias (ux); vT frees the vp bank
                        nc.vector.tensor_copy(out=b_act[:, mc, :],
                                              in_=vp[:, C:C + 1])
                        nc.vector.tensor_copy(out=vT8[:, pair, par, :],
                                              in_=vp[:, 0:C])
                        # exp -> fp8 pT, 1024 cols per instruction
                        for h in range(2):
                            nc.scalar.activation(
                                out=pT8[:, pair, par, h * 1024:(h + 1) * 1024],
                                in_=s[h][:],
                                func=mybir.ActivationFunctionType.Exp,
                                scale=LN2 / 128.0, bias=b_act[:, mc, :])
                        # late w-projection evictions (DVE, one per chunk)
                        for t, oc, xc, mmc, cc_ in late_w:
                            if cc_ == mc:
                                nc.vector.tensor_copy(out=w8[:, oc, xc],
                                                      in_=t[:])
                        # running denominator, query cols 0:1024 (bf16 DVE)
                        src = pT8[:, pair, par, 0:1024]
                        if mc == 0:
                            nc.vector.tensor_copy(out=dacc[:], in_=src)
                        else:
                            nc.vector.tensor_add(dacc[:], dacc[:], src)
                        if par == 1:
                            # pair complete: PE-side denominator for query
                            # cols 1024:2048 and the (2,0) ctx tile; pair 7
                            # closes each accumulation group
                            for q, p0 in ((0, DS2_P0), (1, DS3_P0)):
                                if pair >= p0:
                                    qc = slice(1024 + q * NB,
                                               1024 + (q + 1) * NB)
                                    nc.tensor.matmul(
                                        ds23[q][:], ones8[:],
                                        pT8[:, pair, :, qc],
                                        start=(pair == p0),
                                        stop=(pair == NPAIR - 1),
                                        perf_mode=DR)
                            if pair >= CTX20_P0:
                                nc.tensor.matmul(
                                    ctx20[:], vT8[:, pair, :, 0:P],
                                    pT8[:, pair, :, 1024:1536],
                                    start=(pair == CTX20_P0),
                                    stop=(pair == NPAIR - 1),
                                    perf_mode=DR)


                def ct_evict(ct, qt, cc, nsub, qpick, add_on_dve=False):
                    rows = slice(cc * P, (cc + 1) * P)
                    sub = NB // nsub
                    for si in range(nsub):
                        c0 = qt * NB + si * sub
                        cols = slice(c0, c0 + sub)
                        pcols = slice(si * sub, (si + 1) * sub)
                        t = evict.tile([P, sub], F32, tag="t", name="t")
                        nc.vector.tensor_mul(t[:], ct[:, pcols],
                                             recip[:, cols])
                        o = evict.tile([P, sub], BF16, tag="o", name="o")
                        eng = nc.vector if add_on_dve else nc.gpsimd
                        eng.tensor_add(o[:], t[:], xr[cc][:, cols])
                        deng = (nc.sync, nc.scalar,
                                nc.gpsimd)[(qpick + si) % 3]
                        deng.dma_start(out=out_d[rows, cols], in_=o[:])

                def ctx_mms(ct, qt, cc):
                    for pr in range(NPAIR):
                        nc.tensor.matmul(
                            ct[:],
                            vT8[:, pr, :, cc * P:(cc + 1) * P],
                            pT8[:, pr, :, qt * NB:(qt + 1) * NB],
                            start=(pr == 0), stop=(pr == NPAIR - 1),
                            perf_mode=DR)

                # ---- phase 3a (psCL still open, 5 banks): the first ctx
                # tiles start while the final exps run — pairs 0-6 have no
                # dependence on chunk 15 and keep the PE clock from dropping
                with tc.tile_pool(name="psD1", bufs=1,
                                  space=bass.MemorySpace.PSUM) as psD1:
                    ctx21 = psD1.tile([P, NB], F32, tag="cr1", name="cr21",
                                      bufs=1)
                    ctx_mms(ctx21, 2, 1)
                    # DVE order matters: recip2 then ctx20's recip-multiply
                    # FIRST — it is the last reader of ctx20's PSUM bank
                    nc.vector.reciprocal_approx_fast(out=recip[:, 1024:1536],
                                                     in_=ds23[0][:])
                    t20 = evict.tile([P, NB], F32, tag="t", name="t20")
                    nc.vector.tensor_mul(t20[:], ctx20[:],
                                         recip[:, 1024:1536])
                    nc.vector.reciprocal_approx_fast(out=recip[:, 1536:2048],
                                                     in_=ds23[1][:])
                    # residual prep, pinned behind the denominator so the
                    # scheduler cannot hoist it into the scores-phase queue
                    nc.vector.tensor_scalar(out=bv_late[:], in0=bv_sb[:],
                                            scalar1=ds23[0][:, 0:1],
                                            scalar2=ds23[0][:, 0:1],
                                            op0=mybir.AluOpType.add,
                                            op1=mybir.AluOpType.subtract)
                    for cc in range(2):
                        nc.vector.tensor_scalar_add(out=xr[cc][:],
                                                    in0=xbf[cc][:],
                                                    scalar1=bv_late[:, cc, :])
                    o20 = evict.tile([P, NB], BF16, tag="o", name="o20")
                    nc.gpsimd.tensor_add(o20[:], t20[:], xr[0][:, 1024:1536])
                    nc.sync.dma_start(out=out_d[0:P, 1024:1536], in_=o20[:])

                    # denominator for qt0/qt1 from the bf16 accumulator
                    ds01 = []
                    for q in range(2):
                        cols = slice(q * NB, (q + 1) * NB)
                        ds = psD1.tile([P, NB], F32, tag="ds", name="ds",
                                       bufs=2)
                        ds01.append(ds)
                        nc.tensor.matmul(ds[:], ones_bf[:, 0:P],
                                         dacc[:, cols],
                                         start=True, stop=True)
                    nc.vector.reciprocal_approx_fast(out=recip[:, 0:512],
                                                     in_=ds01[0][:])
                    nc.vector.reciprocal_approx_fast(out=recip[:, 512:1024],
                                                     in_=ds01[1][:])
                    ctx30 = psD1.tile([P, NB], F32, tag="cr2", name="cr30",
                                      bufs=1)
                    ctx_mms(ctx30, 3, 0)
                    ct_evict(ctx21, 2, 1, 1, 1)
                    ctx31 = psD1.tile([P, NB], F32, tag="cr1", name="cr31",
                                      bufs=1)
                    ctx_mms(ctx31, 3, 1)
                    ct_evict(ctx30, 3, 0, 1, 2)
                    # (3,1)'s eviction must trace before psD1 closes so
                    # psD2's tiles inherit the WAR edge on its bank
                    ct_evict(ctx31, 3, 1, 1, 0)

            # ---- phase 3b: remaining ctx tiles + epilogue ----
            with tc.tile_pool(name="psD2", bufs=1,
                              space=bass.MemorySpace.PSUM) as psD2:
                order = ((0, 0), (0, 1), (1, 0), (1, 1))
                evict_after = {
                    (0, 0): [],
                    (0, 1): [((0, 0), 1, 1, False)],
                    (1, 0): [((0, 1), 1, 2, False)],
                    (1, 1): [((1, 0), 2, 0, True)],
                }
                all_tiles = {}
                ctxR = {}
                for qt, cc in order:
                    ct = psD2.tile([P, NB], F32, tag="cr", name="cr",
                                   bufs=4)
                    ctxR[(qt, cc)] = ct
                    all_tiles[(qt, cc)] = ct
                    ctx_mms(ct, qt, cc)
                    for (eqt, ecc), nsub, qpick, dve in evict_after[(qt, cc)]:
                        ct_evict(all_tiles[(eqt, ecc)], eqt, ecc, nsub,
                                 qpick, add_on_dve=dve)
                ct_evict(ctxR[(1, 1)], 1, 1, 4, 2, add_on_dve=True)

    nc.compile()
    return nc


def get_compiled():
    global _COMPILED
    if _COMPILED is None:
        _COMPILED = build_nc()
    return _COMPILED


def make_in_maps(inputs):
    f8 = ml_dtypes.float8_e4m3
    x = np.ascontiguousarray(np.asarray(inputs["x"], dtype=np.float32))
    Wq = np.asarray(inputs["Wq"], np.float32)
    Wk = np.asarray(inputs["Wk"], np.float32)
    Wv = np.asarray(inputs["Wv"], np.float32)
    bq = np.asarray(inputs["bq"], np.float32)
    M = Wq.T @ Wk                               # scores_raw = x^T M x
    u = SCALE * (Wk.T @ bq)                     # per-key score bias u.x
    wvu = np.zeros((C, 272), np.float32)
    wvu[:, 0:C] = Wv.T
    wvu[:, C] = u
    shared = {
        "mt8": np.ascontiguousarray(KAPPA * M.T).astype(f8),
        "wvu8": wvu.astype(f8),
        "bv": np.asarray(inputs["bv"], np.float32).reshape(C, 1),
    }
    return [{"x8": x[i].astype(f8), "xbf": x[i].astype(ml_dtypes.bfloat16),
             **shared} for i in range(B)]


def run(inputs, trace=False, **kwargs):
    nc = get_compiled()
    res = run_bass_kernel_spmd(nc, make_in_maps(inputs),
                               core_ids=list(range(B)), trace=trace, **kwargs)
    out = np.stack([res.results[i]["out"] for i in range(B)], axis=0)
    return out.astype(np.float32), res


def kernel(**inputs):
    out, _ = run(inputs)
    return out


# revision 28
# speedup vs baseline: 1.0568x; 1.0030x over previous
"""Trainium2 Bass kernel for AttentionBlock (B=8, C=256, L=2048), data-parallel
over batch across 8 NeuronCores.

Math (one batch per core, x: [C, L]):
    t^T   = w8^T x8            w8 = fp8(kappa M x),  M = Wq^T Wk,  kappa = 128*SCALE/ln2
    pT    = exp(t*ln2/128 + ux)  [m, l], keys m on partitions, fp8 direct from ACT
    denom = per-query sum of pT: cols 0:1024 via bf16 DVE accumulator + ones
            matmul, cols 1024:2048 via fp8 DoubleRow ones-matmuls in PSUM
    ctx   = vT^T pT in fp8 DoubleRow over chunk PAIRS (256 keys/instruction)
    out   = ctx * (1/denom) + (bf16(x) + bv)

Schedule (v10):
  - exp: 2 x 1024-wide ACT instructions per chunk writing fp8 pT directly;
    ACT paces phase 2 at ~2.26us/chunk
  - x8 loads ride FIVE dma queues (sync/scalar/gpsimd/vector/tensor) in
    256-col slices so all of x8 is resident by ~11.5us; xbf rides last on
    the scalar queue (coarse per-queue completion semaphores would
    otherwise stall x8 consumers behind the 1MB xbf transfer)
  - no separate w-projection PSUM pool: cols 0:1024 go through two score
    tile rotations before chunk 0; cols 1024:2048 borrow the ds2/ds3/ctx20
    banks as scratch during chunks 1-4 (their accumulations start at pairs
    3/2/1 and the missed pairs are caught up at the start of phase 3)
  - PSUM: scores 2x[P,1024] (4) + vp (1) + ds2/ds3 (2) + ctx(2,0) (1)
  - phase 3: catch-up mms, denominator finish, 7 remaining ctx tiles in
    fp8 DR with evictions interleaved; tail tiles' residual adds on DVE
    (gpsimd tensor_add is ~1.1us/512) and the last tile evicts in 4
    sub-slices across 3 dma queues
"""

import math
import numpy as np
import ml_dtypes

import concourse.bass as bass
import concourse.tile as tile
from concourse import bacc, mybir
from concourse.bass_utils import run_bass_kernel_spmd

B, C, L = 8, 256, 2048
P = 128                 # partitions
NMC = L // P            # 16 m-chunks (key blocks)
NPAIR = NMC // 2        # 8 key pairs (256 keys each)
NB = 512                # matmul moving free dim
SCALE = float(C) ** -0.5
LN2 = math.log(2.0)
KAPPA = 128.0 * SCALE / LN2     # scores t = kappa * s_raw (baked into mt8 on host)

F32 = mybir.dt.float32
BF16 = mybir.dt.bfloat16
F8 = mybir.dt.float8e4
DR = mybir.MatmulPerfMode.DoubleRow

# first phase-2 pair each accumulator covers (earlier pairs caught up in
# phase 3); constrained by when each tile is free of w-projection scratch
CTX20_P0, DS2_P0, DS3_P0 = 1, 3, 2

_COMPILED = None


def build_nc():
    nc = bacc.Bacc("TRN2", target_bir_lowering=False, debug=False, num_devices=8)

    x8_d = nc.dram_tensor("x8", [C, L], F8, kind="ExternalInput").ap()
    xbf_d = nc.dram_tensor("xbf", [C, L], BF16, kind="ExternalInput").ap()
    mt8_d = nc.dram_tensor("mt8", [C, C], F8, kind="ExternalInput").ap()
    wvu8_d = nc.dram_tensor("wvu8", [C, 272], F8, kind="ExternalInput").ap()
    bv_d = nc.dram_tensor("bv", [C, 1], F32, kind="ExternalInput").ap()
    out_d = nc.dram_tensor("out", [C, L], BF16, kind="ExternalOutput").ap()

    with tile.TileContext(nc) as tc:
        with (
            tc.tile_pool(name="const", bufs=1) as const,
            tc.tile_pool(name="data", bufs=1) as data,
            tc.tile_pool(name="evict", bufs=6) as evict,
        ):
            # ---- constants ----
            ones_bf = const.tile([P, NB], BF16)
            nc.vector.memset(ones_bf[:], 1.0)
            ones8 = const.tile([P, 2, P], F8)
            nc.gpsimd.memset(ones8[:], 1.0)
            tiny = const.tile([P, 4, 16], F32)

            x8 = data.tile([P, 2, L], F8, tag="x8", name="x8")
            xbf = [data.tile([P, L], BF16, tag=f"xbf{c}", name=f"xbf{c}")
                   for c in range(2)]
            mt8 = const.tile([P, 2, C], F8, tag="mt8")
            wvu8 = const.tile([P, 2, 272], F8, tag="wvu8")
            bv_sb = const.tile([P, 2, 1], F32, tag="bv")

            def x8_dma(j, c0, c1, eng):
                # per-j slices keep DRAM reads row-contiguous: [128 rows,
                # c1-c0 bytes] per transfer. Column-slicing BOTH j halves
                # would cut packets to (c1-c0)/2... no — j-split is what
                # keeps each descriptor a single contiguous DRAM run.
                eng.dma_start(out=x8[:, j, c0:c1],
                              in_=x8_d[j * P:(j + 1) * P, c0:c1])

            # x8 loads: a narrow column head (cols 0:512, 512B packets) lets
            # the w projection start early; the remaining cols 512:2048 move
            # as two row-major 192KB transfers (1.5KB packets, ~2x the
            # per-queue rate of column slices). Weights first on scalar.
            # bv/xbf descriptor generation is deferred (traced after the w8
            # evicts) and xbf rides last on the scalar queue: the per-queue
            # completion semaphores are coarse, so x8 consumers would stall
            # behind the 1MB xbf transfer otherwise.
            nc.scalar.dma_start(out=mt8[:],
                                in_=mt8_d.rearrange("(j p) o -> p j o", p=P))
            x8_dma(0, 0, 512, nc.gpsimd)
            x8_dma(1, 0, 512, nc.sync)
            nc.scalar.dma_start(out=wvu8[:],
                                in_=wvu8_d.rearrange("(j p) o -> p j o", p=P))
            x8_dma(0, 512, 2048, nc.gpsimd)
            x8_dma(1, 512, 2048, nc.sync)

            w8 = data.tile([P, 2, L], F8, tag="w8", name="w8")
            vT8 = data.tile([P, NPAIR, 2, C], F8, tag="vT8")
            pT8 = data.tile([P, NPAIR, 2, L], F8, tag="pT8")
            b_act = data.tile([P, NMC, 1], F32, tag="b_act")
            bv_late = data.tile([P, 2, 1], F32, tag="bv_late")
            dacc = data.tile([P, 1024], BF16, tag="dacc")
            recip = data.tile([P, L], F32, tag="recip")
            xr = [data.tile([P, L], BF16, tag=f"xr{c}", name=f"xr{c}")
                  for c in range(2)]

            with tc.tile_pool(name="psCL", bufs=1,
                              space=bass.MemorySpace.PSUM) as psCL:
                ctx20 = psCL.tile([P, NB], F32, tag="c20", name="c20", bufs=1)
                ds23 = [psCL.tile([P, NB], F32, tag=f"ds{q}", name=f"ds{q}",
                                  bufs=1) for q in (2, 3)]

                with tc.tile_pool(name="psB", bufs=1,
                                  space=bass.MemorySpace.PSUM) as psB:
                    # ---- warmup: ACT/DVE table loads + PE spin-up; matmul
                    # targets are overwritten later via start=True ----
                    nc.vector.memset(tiny[:, 0, :], 1.0)
                    nc.scalar.activation(out=tiny[:, 1, :], in_=tiny[:, 0, :],
                                         func=mybir.ActivationFunctionType.Exp,
                                         scale=1.0)
                    nc.vector.reciprocal_approx_fast(out=tiny[:, 2, :],
                                                     in_=tiny[:, 0, :])
                    for i in range(3):
                        nc.tensor.matmul((ds23 + [ctx20])[i][:],
                                         ones_bf[:, 0:P], ones_bf[:],
                                         start=True, stop=True)
                    nc.tensor.matmul(ds23[0][0:32, 0:16], ones8[:, :, 0:32],
                                     ones8[:, :, 0:16], start=True, stop=True,
                                     perf_mode=DR)

                    # ---- w projection cols 0:1024 through two score-tile
                    # rotations (w = kappa M x, DoubleRow, 256 deep) ----
                    wpA = [psB.tile([P, 1024], F32, tag="s", name=f"wpA{oc}",
                                    bufs=2) for oc in range(2)]

                    def wmm(dst, dcols, oc, xcols):
                        nc.tensor.matmul(dst[:, dcols],
                                         mt8[:, :, oc * P:(oc + 1) * P],
                                         x8[:, :, xcols],
                                         start=True, stop=True, perf_mode=DR)

                    # evicts trace right after their block's matmuls: the
                    # completion semaphores are coarse (threshold = all PE
                    # work traced so far), so an evict traced after ALL four
                    # matmuls waits for all four. The ACT engine's dispatch
                    # around this window is erratic, so it only gets the
                    # first copy; DVE (predictable in-order) takes the rest.
                    for b in range(2):
                        cols = slice(b * NB, (b + 1) * NB)
                        for oc in range(2):
                            wmm(wpA[oc], cols, oc, cols)
                        if b == 0:
                            nc.scalar.copy(out=w8[:, 0, cols],
                                           in_=wpA[0][:, cols])
                        else:
                            nc.vector.tensor_copy(out=w8[:, 0, cols],
                                                  in_=wpA[0][:, cols])
                        nc.vector.tensor_copy(out=w8[:, 1, cols],
                                              in_=wpA[1][:, cols])
                    # deferred descriptor generation (data needed in phase 3)
                    nc.scalar.dma_start(
                        out=bv_sb[:],
                        in_=bv_d.rearrange("(j p) o -> p j o", p=P))
                    nc.scalar.dma_start(out=xbf[0][:], in_=xbf_d[0:P, :])
                    nc.scalar.dma_start(out=xbf[1][:], in_=xbf_d[P:C, :])

                    # late w-projection (cols 1024:2048) borrows psCL banks as
                    # scratch: (tile, psum cols, w8 oc, x8 cols, mm chunk,
                    # cast chunk)
                    late_w = [
                        (ds23[0], 0, slice(1024, 1536), 1, 1),
                        (ds23[1], 1, slice(1024, 1536), 1, 2),
                        (ctx20, 0, slice(1536, 2048), 2, 3),
                        (ds23[0], 1, slice(1536, 2048), 4, 5),
                    ]

                    for mc in range(NMC):
                        pair, par = mc // 2, mc % 2
                        mrows = slice(mc * P, (mc + 1) * P)
                        # v/ux projection for this key chunk
                        vp = psB.tile([P, 272], F32, tag="vp", name="vp",
                                      bufs=1)
                        nc.tensor.matmul(
                            vp[:], x8[:, :, mrows], wvu8[:],
                            start=True, stop=True, perf_mode=DR)
                        # scores, two 1024-wide tiles (2 x 512 mms each)
                        s = [psB.tile([P, 1024], F32, tag="s", name="s",
                                      bufs=2) for _ in range(2)]
                        for h in range(2):
                            for ln in range(2):
                                q0 = h * 1024 + ln * NB
                                nc.tensor.matmul(
                                    s[h][:, ln * NB:(ln + 1) * NB],
                                    w8[:, :, mrows],
                                    x8[:, :, q0:q0 + NB],
                                    start=True, stop=True, perf_mode=DR)
                        # late w-projection matmuls scheduled for this chunk
                        for t, oc, xc, mmc, cc_ in late_w:
                            if mmc == mc:
                                wmm(t, slice(0, NB), oc, xc)
                        # catch-up mms for the pairs the accumulators missed
                        # while holding w-projection scratch. Traced at the
                        # top of chunk 15, before its exps: they fill the PE
                        # idle window while pair-7 work waits on the final
                        # exps — a >1us PE stall here would drop the PE clock
                        # to its mid p-state for the next ~3us of phase 3.
                        if mc == 15:
                            for q, prs in ((0, (0, 1, 2)), (1, (0, 1))):
                                qc = slice(1024 + q * NB,
                                           1024 + (q + 1) * NB)
                                for pr in prs:
                                    nc.tensor.matmul(ds23[q][:], ones8[:],
                                                     pT8[:, pr, :, qc],
                                                     start=False, stop=False,
                                                     perf_mode=DR)
                            nc.tensor.matmul(ctx20[:], vT8[:, 0, :, 0:P],
                                             pT8[:, 0, :, 1024:1536],
                                             start=False, stop=False,
                                             perf_mode=DR)
                        # per-key exp bias (ux); vT frees the vp bank
                        nc.vector.tensor_copy(out=b_act[:, mc, :],
                                              in_=vp[:, C:C + 1])
                        nc.vector.tensor_copy(out=vT8[:, pair, par, :],
                                              in_=vp[:, 0:C])
                        # exp -> fp8 pT, 1024 cols per instruction
                        for h in range(2):
                            nc.scalar.activation(
                                out=pT8[:, pair, par, h * 1024:(h + 1) * 1024],
                                in_=s[h][:],
                                func=mybir.ActivationFunctionType.Exp,
                                scale=LN2 / 128.0, bias=b_act[:, mc, :])
                        # late w-projection evictions (DVE, one per chunk)
                        for t, oc, xc, mmc, cc_ in late_w:
                            if cc_ == mc:
                                nc.vector.tensor_copy(out=w8[:, oc, xc],
                                                      in_=t[:])
                        # running denominator, query cols 0:1024 (bf16 DVE)
                        src = pT8[:, pair, par, 0:1024]
                        if mc == 0:
                            nc.vector.tensor_copy(out=dacc[:], in_=src)
                        else:
                            nc.vector.tensor_add(dacc[:], dacc[:], src)
                        if par == 1:
                            # pair complete: PE-side denominator for query
                            # cols 1024:2048 and the (2,0) ctx tile; pair 7
                            # closes each accumulation group
                            for q, p0 in ((0, DS2_P0), (1, DS3_P0)):
                                if pair >= p0:
                                    qc = slice(1024 + q * NB,
                                               1024 + (q + 1) * NB)
                                    nc.tensor.matmul(
                                        ds23[q][:], ones8[:],
                                        pT8[:, pair, :, qc],
                                        start=(pair == p0),
                                        stop=(pair == NPAIR - 1),
                                        perf_mode=DR)
                            if pair >= CTX20_P0:
                                nc.tensor.matmul(
                                    ctx20[:], vT8[:, pair, :, 0:P],
                                    pT8[:, pair, :, 1024:1536],
                                    start=(pair == CTX20_P0),
                                    stop=(pair == NPAIR - 1),
                                    perf_mode=DR)


                def ct_evict(ct, qt, cc, nsub, qpick, add_on_dve=False):
                    rows = slice(cc * P, (cc + 1) * P)
                    sub = NB // nsub
                    for si in range(nsub):
                        c0 = qt * NB + si * sub
                        cols = slice(c0, c0 + sub)
                        pcols = slice(si * sub, (si + 1) * sub)
                        t = evict.tile([P, sub], F32, tag="t", name="t")
                        nc.vector.tensor_mul(t[:], ct[:, pcols],
                                             recip[:, cols])
                        o = evict.tile([P, sub], BF16, tag="o", name="o")
                        eng = nc.vector if add_on_dve else nc.gpsimd
                        eng.tensor_add(o[:], t[:], xr[cc][:, cols])
                        deng = (nc.sync, nc.scalar,
                                nc.gpsimd)[(qpick + si) % 3]
                        deng.dma_start(out=out_d[rows, cols], in_=o[:])

                def ctx_mms(ct, qt, cc):
                    for pr in range(NPAIR):
                        nc.tensor.matmul(
                            ct[:],
                            vT8[:, pr, :, cc * P:(cc + 1) * P],
                            pT8[:, pr, :, qt * NB:(qt + 1) * NB],
                            start=(pr == 0), stop=(pr == NPAIR - 1),
                            perf_mode=DR)

                # ---- phase 3a (psCL still open, 5 banks): the first ctx
                # tiles start while the final exps run — pairs 0-6 have no
                # dependence on chunk 15 and keep the PE clock from dropping
                with tc.tile_pool(name="psD1", bufs=1,
                                  space=bass.MemorySpace.PSUM) as psD1:
                    ctx21 = psD1.tile([P, NB], F32, tag="cr1", name="cr21",
                                      bufs=1)
                    ctx_mms(ctx21, 2, 1)
                    # DVE order matters: recip2 then ctx20's recip-multiply
                    # FIRST — it is the last reader of ctx20's PSUM bank
                    nc.vector.reciprocal_approx_fast(out=recip[:, 1024:1536],
                                                     in_=ds23[0][:])
                    t20 = evict.tile([P, NB], F32, tag="t", name="t20")
                    nc.vector.tensor_mul(t20[:], ctx20[:],
                                         recip[:, 1024:1536])
                    nc.vector.reciprocal_approx_fast(out=recip[:, 1536:2048],
                                                     in_=ds23[1][:])
                    # residual prep, pinned behind the denominator so the
                    # scheduler cannot hoist it into the scores-phase queue
                    nc.vector.tensor_scalar(out=bv_late[:], in0=bv_sb[:],
                                            scalar1=ds23[0][:, 0:1],
                                            scalar2=ds23[0][:, 0:1],
                                            op0=mybir.AluOpType.add,
                                            op1=mybir.AluOpType.subtract)
                    for cc in range(2):
                        nc.vector.tensor_scalar_add(out=xr[cc][:],
                                                    in0=xbf[cc][:],
                                                    scalar1=bv_late[:, cc, :])
                    o20 = evict.tile([P, NB], BF16, tag="o", name="o20")
                    nc.gpsimd.tensor_add(o20[:], t20[:], xr[0][:, 1024:1536])
                    nc.sync.dma_start(out=out_d[0:P, 1024:1536], in_=o20[:])

                    # denominator for qt0/qt1 from the bf16 accumulator
                    ds01 = []
                    for q in range(2):
                        cols = slice(q * NB, (q + 1) * NB)
                        ds = psD1.tile([P, NB], F32, tag="ds", name="ds",
                                       bufs=2)
                        ds01.append(ds)
                        nc.tensor.matmul(ds[:], ones_bf[:, 0:P],
                                         dacc[:, cols],
                                         start=True, stop=True)
                    nc.vector.reciprocal_approx_fast(out=recip[:, 0:512],
                                                     in_=ds01[0][:])
                    nc.vector.reciprocal_approx_fast(out=recip[:, 512:1024],
                                                     in_=ds01[1][:])
                    ctx30 = psD1.tile([P, NB], F32, tag="cr2", name="cr30",
                                      bufs=1)
                    ctx_mms(ctx30, 3, 0)
                    ct_evict(ctx21, 2, 1, 1, 1)
                    ctx31 = psD1.tile([P, NB], F32, tag="cr1", name="cr31",
                                      bufs=1)
                    ctx_mms(ctx31, 3, 1)
                    ct_evict(ctx30, 3, 0, 1, 2)
                    # (3,1)'s eviction must trace before psD1 closes so
                    # psD2's tiles inherit the WAR edge on its bank
                    ct_evict(ctx31, 3, 1, 1, 0)

            # ---- phase 3b: remaining ctx tiles + epilogue ----
            with tc.tile_pool(name="psD2", bufs=1,
                              space=bass.MemorySpace.PSUM) as psD2:
                order = ((0, 0), (0, 1), (1, 0), (1, 1))
                evict_after = {
                    (0, 0): [],
                    (0, 1): [((0, 0), 1, 1, False)],
                    (1, 0): [((0, 1), 1, 2, False)],
                    (1, 1): [((1, 0), 2, 0, True)],
                }
                all_tiles = {}
                ctxR = {}
                for qt, cc in order:
                    ct = psD2.tile([P, NB], F32, tag="cr", name="cr",
                                   bufs=4)
                    ctxR[(qt, cc)] = ct
                    all_tiles[(qt, cc)] = ct
                    ctx_mms(ct, qt, cc)
                    for (eqt, ecc), nsub, qpick, dve in evict_after[(qt, cc)]:
                        ct_evict(all_tiles[(eqt, ecc)], eqt, ecc, nsub,
                                 qpick, add_on_dve=dve)
                ct_evict(ctxR[(1, 1)], 1, 1, 4, 2, add_on_dve=True)

    nc.compile()
    return nc


def get_compiled():
    global _COMPILED
    if _COMPILED is None:
        _COMPILED = build_nc()
    return _COMPILED


def make_in_maps(inputs):
    f8 = ml_dtypes.float8_e4m3
    x = np.ascontiguousarray(np.asarray(inputs["x"], dtype=np.float32))
    Wq = np.asarray(inputs["Wq"], np.float32)
    Wk = np.asarray(inputs["Wk"], np.float32)
    Wv = np.asarray(inputs["Wv"], np.float32)
    bq = np.asarray(inputs["bq"], np.float32)
    M = Wq.T @ Wk                               # scores_raw = x^T M x
    u = SCALE * (Wk.T @ bq)                     # per-key score bias u.x
    wvu = np.zeros((C, 272), np.float32)
    wvu[:, 0:C] = Wv.T
    wvu[:, C] = u
    shared = {
        "mt8": np.ascontiguousarray(KAPPA * M.T).astype(f8),
        "wvu8": wvu.astype(f8),
        "bv": np.asarray(inputs["bv"], np.float32).reshape(C, 1),
    }
    return [{"x8": x[i].astype(f8), "xbf": x[i].astype(ml_dtypes.bfloat16),
             **shared} for i in range(B)]


def run(inputs, trace=False, **kwargs):
    nc = get_compiled()
    res = run_bass_kernel_spmd(nc, make_in_maps(inputs),
                               core_ids=list(range(B)), trace=trace, **kwargs)
    out = np.stack([res.results[i]["out"] for i in range(B)], axis=0)
    return out.astype(np.float32), res


def kernel(**inputs):
    out, _ = run(inputs)
    return out
